# revision 1
# baseline (speedup 1.0000x reference)
"""Trainium2 Bass kernel for nn_Decoding_Layer (dense transformer decoder layer).

Sharding: 8 cores = 4 batches x 2 sequence-halves. Each core computes one
512-row query block of one batch end-to-end (no collectives). K/V projections
are computed over the full 1024-key sequence per core; causal masking is
data-driven (host-fed mask slice for the diagonal 512x512 block plus a V-row
mask that zeroes invalid key blocks), so all 8 cores run one uniform program.

All big matmuls run as float32r (fp32 operands truncated to ~fp22 inside the
PE at full bf16-rate) with fp32 PSUM accumulation. Activations are kept
feature-major ("transposed", [feat, row]) so weights load untransposed and
per-feature biases fold into per-partition ACT bias slots during PSUM drains.
"""

import sys

if "/opt/trn_rl_repo" not in sys.path:
    sys.path.insert(0, "/opt/trn_rl_repo")

import numpy as np

import concourse.bass as bass
import concourse.mybir as mybir
import concourse.tile as tile
from concourse import bass_utils
from concourse.bass_utils import run_bass_kernel_spmd

# walrus ships with --enable-ldw-opt=false; enabling it lets codegen overlap
# the per-matmul 4-byte weight loads, which otherwise serialize with the
# matmul stream on this fp32r-heavy kernel.
_orig_run_command = bass_utils.run_command

def _patched_run_command(argv, **kw):
    argv = ["--enable-ldw-opt=true" if a == "--enable-ldw-opt=false" else a
            for a in argv]
    return _orig_run_command(argv, **kw)

bass_utils.run_command = _patched_run_command

f32 = mybir.dt.float32
f32r = mybir.dt.float32r
AF = mybir.ActivationFunctionType
ALU = mybir.AluOpType

B, S, D, H, DFF = 4, 1024, 1024, 16, 4096
DEPTH = D // H
R = 512          # rows (query block) per core
EPS = 1e-6
N_CORES = 8


def _split_waits(nc, maxw=1):
    """Walrus in this toolchain encodes at most one semaphore wait per
    instruction; Tile emits several. Move excess waits onto same-engine NOPs
    placed immediately before the instruction (sequential per-engine streams
    make this equivalent)."""
    for f in nc.m.functions:
        for bb in f.blocks:
            out = []
            for inst in bb.instructions:
                si = inst.sync_info
                if si is not None and len(si.on_wait) > maxw:
                    waits = list(si.on_wait)
                    keep, excess = waits[-maxw:], waits[:-maxw]
                    eng = getattr(inst, "engine", None)
                    k = 0
                    while excess:
                        chunk, excess = excess[:maxw], excess[maxw:]
                        out.append(mybir.InstNoOp(
                            name=f"{inst.name}_wsp{k}",
                            engine=eng,
                            bass_nofuse=True,
                            sync_info=mybir.SyncInfo(on_wait=chunk, on_update=[]),
                        ))
                        k += 1
                    inst.sync_info = mybir.SyncInfo(
                        on_wait=keep, on_update=list(si.on_update))
                out.append(inst)
            bb.instructions = out


def build_program():
    nc = bass.Bass("TRN2", target_bir_lowering=False, debug=False)

    def din(name, shape):
        return nc.dram_tensor(name, shape, f32, kind="ExternalInput").ap()

    dc_own_d = din("dc_own", [D, R])        # dec_input own rows, transposed
    dke_d = din("dke", [D, S])              # dec keys (reordered: ctx|diag), transposed
    enc_d = din("encT", [D, S])             # enc_output, transposed
    mask_d = din("maskT", [R, R])           # causal diag block, [key, q], pre * -8e9
    padb_d = din("padb", [128, 8])          # -1e9 * padding_mask, chunked
    vm1_d = din("vm1", [128, 8])            # self V-row mask (chunked)
    vm1r_d = din("vm1r", [128, 8, 16])      # same, replicated per head
    vm2_d = din("vm2", [128, 8])            # ones
    vm2r_d = din("vm2r", [128, 8, 16])      # ones
    ones_d = din("onesd", [128, 128])       # ones
    w_d = {k: din(k, [D, D]) for k in ("wq1", "wk1", "wq2", "wk2", "wo1", "wo2")}
    fw1_d = din("fw1", [D, DFF])
    fw2_d = din("fw2", [DFF, D])
    bc_d = {k: din(k, [128, 8]) for k in
            ("bq1c", "bk1c", "bq2c", "bk2c", "bo1c", "bo2c", "fb2c",
             "g1c", "b1c", "g2c", "b2c", "g3c", "b3c")}
    fb1c_d = din("fb1c", [128, 32])
    out_d = nc.dram_tensor("outT", [D, R], f32, kind="ExternalOutput").ap()

    with tile.TileContext(nc) as tc:
        with tc.tile_pool(name="persist", bufs=1) as pp, \
             tc.tile_pool(name="consts", bufs=1) as cp:
            # ---- persistent SBUF ----
            arena = pp.tile([128, 16384], f32r, name="arena")     # 8 MiB
            dke = arena[:, 0:8192].rearrange("p (f r) -> p f r", f=8)
            kTv = arena[:, 8192:16384].rearrange("p (f r) -> p f r", f=8)
            vaug = pp.tile([128, 8, 16, 65], f32r, name="vaug")
            dc_own = pp.tile([128, 8, R], f32r, name="dc_own")
            qbuf = pp.tile([128, 8, R], f32r, name="qbuf")        # q1T -> q2T
            abufA = pp.tile([128, 8, R], f32r, name="abufA")      # attn1T -> x2pre/x2T
            xa = pp.tile([128, 8, R], f32r, name="xa")            # x1pre/x1T -> x3pre
            # phase-3/4 views of the arena: enc goes where k1T lived, k2T where
            # dec-keys lived, attn2T into the (then-dead) enc region.
            kT2v = arena[:, 0:8192].rearrange("p (f r) -> p f r", f=8)
            encv = arena[:, 8192:16384].rearrange("p (f r) -> p f r", f=8)
            abufB = arena[:, 8192:12288].rearrange("p (f r) -> p f r", f=8)

            # ---- constants ----
            onesb = cp.tile([128, 128], f32r, name="onesb")
            padb = cp.tile([128, 8], f32, name="padb")
            vm1 = cp.tile([128, 8], f32, name="vm1")
            vm2 = cp.tile([128, 8], f32, name="vm2")
            bcs = {k: cp.tile([128, 8], f32, name=k) for k in bc_d}
            fb1c = cp.tile([128, 32], f32, name="fb1c")

            # critical-path loads first on the SP queue: warmup needs onesb +
            # dc_own chunk 0; the first q1T drain needs bq1c. Everything else
            # rides the otherwise-idle GpSimd SWDGE queues.
            nc.sync.dma_start(out=onesb, in_=ones_d.bitcast(f32r))
            dco_r = dc_own_d.rearrange("(f p) r -> p f r", p=128).bitcast(f32r)
            for kc in range(8):
                nc.sync.dma_start(out=dc_own[:, kc, :], in_=dco_r[:, kc, :])
            nc.sync.dma_start(out=bcs["bq1c"], in_=bc_d["bq1c"])
            nc.scalar.dma_start(out=dke,
                                in_=dke_d.rearrange("(f p) r -> p f r", p=128).bitcast(f32r))
            nc.gpsimd.dma_start(out=padb, in_=padb_d)
            nc.gpsimd.dma_start(out=vm1, in_=vm1_d)
            nc.gpsimd.dma_start(out=vm2, in_=vm2_d)
            for k in bcs:
                if k != "bq1c":
                    nc.gpsimd.dma_start(out=bcs[k], in_=bc_d[k])
            nc.gpsimd.dma_start(out=fb1c, in_=fb1c_d)

            ones1 = onesb[0:1, :]      # [1, 128] f32r
            onesp = onesb[:, 0:1]      # [128, 1] f32r

            # one kernel-lifetime weight-stream pool: per-phase pools would
            # re-allocate SBUF at each phase boundary, stalling the prefetch.
            wall = ctx_wp = tc.tile_pool(name="wall", bufs=6)
            wall = wall.__enter__()

            # ~10us of dummy matmuls while the first DMAs land: pulls the PE
            # HAM clock-gate to 8/8 before the real stream begins.
            with tc.tile_pool(name="warm", bufs=1, space="PSUM") as wps:
                wtile = wps.tile([128, 512], f32, name="warm")
                for wi in range(24):
                    nc.tensor.matmul(
                        wtile[:],
                        lhsT=onesb[:, 0:128],
                        rhs=dc_own[:, 0, :],
                        start=(wi == 0), stop=(wi == 23))

            # ---- helpers ----
            def gemm_TN(Wd, xt, KCn, MCn, NN, drain, wp, ps):
                """OUT^T[m-chunk, n] = sum_kc W[kc, m]^T @ xt(kc, n).
                xt(kc, n) -> [128, 512] f32r AP. drain(mi, n, psum_ap)."""
                g = max(1, 4 // NN)
                for mg in range(0, MCn, g):
                    gs = min(g, MCn - mg)
                    pps = {}
                    for i in range(gs):
                        for n in range(NN):
                            pps[(i, n)] = ps.tile([128, 512], f32, name="pp")
                    for kc in range(KCn):
                        wt = wp.tile([128, gs * 128], f32r, name="wt")
                        nc.sync.dma_start(
                            out=wt,
                            in_=Wd[kc * 128:(kc + 1) * 128,
                                   mg * 128:(mg + gs) * 128].bitcast(f32r))
                        for i in range(gs):
                            for n in range(NN):
                                nc.tensor.matmul(
                                    pps[(i, n)][:],
                                    lhsT=wt[:, i * 128:(i + 1) * 128],
                                    rhs=xt(kc, n),
                                    start=(kc == 0), stop=(kc == KCn - 1))
                    for i in range(gs):
                        for n in range(NN):
                            drain(mg + i, n, pps[(i, n)])

            def gemm_NT(Wd, xt_sb, KCn, RCn, NFn, drain, wp, ps):
                """OUT[r-chunk] = X @ W : lhsT = xt chunks, rhs = W cols.
                drain(rc, nf, psum_ap). xt_sb [128, KCn, S] f32r.
                Weight stream rides the ACT HWDGE queue to offload SP."""
                for nf in range(NFn):
                    for rg in range(0, RCn, 4):
                        gs = min(4, RCn - rg)
                        pps = [ps.tile([128, 512], f32, name="pp") for _ in range(gs)]
                        for kc in range(KCn):
                            wt = wp.tile([128, 512], f32r, name="wt")
                            nc.scalar.dma_start(
                                out=wt,
                                in_=Wd[kc * 128:(kc + 1) * 128,
                                       nf * 512:(nf + 1) * 512].bitcast(f32r))
                            for i in range(gs):
                                nc.tensor.matmul(
                                    pps[i][:],
                                    lhsT=xt_sb[:, kc, (rg + i) * 128:(rg + i + 1) * 128],
                                    rhs=wt[:],
                                    start=(kc == 0), stop=(kc == KCn - 1))
                        for i in range(gs):
                            drain(rg + i, nf, pps[i])

            def attention(q_sb, kT_sb, v_sb, attn_out, is_self, masks, spool,
                          epool, upool, ps_s, ps_av, ps_b):
                for f in range(8):
                    avs = [ps_av.tile([65, 512], f32, name="av") for _ in range(2)]
                    for kc in range(8):
                        ss = ps_s.tile([128, 1024], f32, name="ss")
                        for a in range(2):
                            nc.tensor.matmul(
                                ss[:, a * 512:(a + 1) * 512],
                                lhsT=kT_sb[64 * a:64 * (a + 1), f, kc * 128:(kc + 1) * 128],
                                rhs=q_sb[64 * a:64 * (a + 1), f, :],
                                start=True, stop=True)
                        if is_self and kc >= 4:
                            nc.vector.tensor_tensor(
                                out=ss[:].rearrange("p (a r) -> p a r", a=2),
                                in0=ss[:].rearrange("p (a r) -> p a r", a=2),
                                in1=bass.AP(tensor=masks.tensor,
                                            offset=masks[:, kc - 4, :].offset,
                                            ap=[list(masks.ap[0]), [0, 2],
                                                list(masks.ap[2])]),
                                op=ALU.add)
                        e = epool.tile([128, 1024], f32r, name="ee")
                        bias = 0.0 if is_self else padb[:, kc:kc + 1]
                        nc.scalar.activation(e[:], ss[:], AF.Exp,
                                             bias=bias, scale=0.125)
                        for a in range(2):
                            nc.tensor.matmul(
                                avs[a][:],
                                lhsT=v_sb[:, kc, 2 * f + a, :],
                                rhs=e[:, a * 512:(a + 1) * 512],
                                start=(kc == 0), stop=(kc == 7))
                    for a in range(2):
                        rec = upool.tile([1, 512], f32r, name="rec")
                        with nc.allow_low_precision(reason="f32r keeps fp32 bits"):
                            nc.vector.reciprocal(rec[:], avs[a][64:65, :])
                        dst = attn_out[64 * a:64 * (a + 1), f, :]
                        nc.scalar.copy(dst, avs[a][0:64, :])
                        bp = ps_b.tile([64, 512], f32, name="bp")
                        nc.tensor.matmul(bp[:], lhsT=onesb[0:1, 0:64], rhs=rec[:],
                                         start=True, stop=True)
                        with nc.allow_low_precision(reason="f32r keeps fp32 bits"):
                            nc.vector.tensor_mul(dst, dst, bp[:])

            def layernorm(x_sb, gC, bC, out_sb, sqp, ltp, ps_ln):
                pm = ps_ln.tile([1, 512], f32, name="pm")
                pv = ps_ln.tile([1, 512], f32, name="pv")
                for kc in range(8):
                    nc.tensor.matmul(pm[:], lhsT=onesp, rhs=x_sb[:, kc, :],
                                     start=(kc == 0), stop=(kc == 7))
                    sq = sqp.tile([128, 512], f32r, name="sq")
                    nc.scalar.activation(sq[:], x_sb[:, kc, :], AF.Square)
                    nc.tensor.matmul(pv[:], lhsT=onesp, rhs=sq[:],
                                     start=(kc == 0), stop=(kc == 7))
                m = ltp.tile([1, 512], f32, name="lm")
                sc = ltp.tile([1, 512], f32, name="lsc")
                sc2 = ltp.tile([1, 512], f32, name="lsc2")
                inv = ltp.tile([1, 512], f32r, name="linv")
                minv = ltp.tile([1, 512], f32r, name="lminv")
                nc.vector.tensor_scalar_mul(m[:], pm[:], 1.0 / D)
                nc.vector.tensor_scalar_mul(sc[:], pv[:], 1.0 / D)   # E[x^2]
                nc.vector.tensor_mul(sc2[:], m[:], m[:])             # m^2
                nc.vector.tensor_scalar_add(sc2[:], sc2[:], -EPS)
                nc.vector.tensor_tensor(out=sc[:], in0=sc[:], in1=sc2[:],
                                        op=ALU.subtract)             # var + eps
                nc.scalar.activation(sc[:], sc[:], AF.Sqrt)
                with nc.allow_low_precision(reason="f32r keeps fp32 bits"):
                    nc.vector.reciprocal(inv[:], sc[:])
                    nc.vector.tensor_mul(minv[:], m[:], inv[:])
                binv = ps_ln.tile([128, 512], f32, name="binv")
                bmv = ps_ln.tile([128, 512], f32, name="bmv")
                nc.tensor.matmul(binv[:], lhsT=ones1, rhs=inv[:], start=True, stop=True)
                nc.tensor.matmul(bmv[:], lhsT=ones1, rhs=minv[:], start=True, stop=True)
                def bc8(ps_t):
                    return bass.AP(tensor=ps_t.tensor, offset=ps_t.offset,
                                   ap=[list(ps_t.ap[0]), [0, 8], list(ps_t.ap[1])])
                with nc.allow_low_precision(reason="f32r keeps fp32 bits"):
                    nc.vector.tensor_tensor(out=x_sb[:], in0=x_sb[:], in1=bc8(binv),
                                            op=ALU.mult)
                    nc.vector.tensor_tensor(out=x_sb[:], in0=x_sb[:], in1=bc8(bmv),
                                            op=ALU.subtract)
                for kc in range(8):
                    nc.scalar.activation(out_sb[:, kc, :], x_sb[:, kc, :], AF.Identity,
                                         bias=bC[:, kc:kc + 1], scale=gC[:, kc:kc + 1])

            # ================= phase 1: self projections =================
            with tc.tile_pool(name="ps1", bufs=6, space="PSUM") as ps:
                wp = wall

                def drain_q1(mi, n, pa):
                    nc.scalar.activation(qbuf[:, mi, :], pa[:], AF.Identity,
                                         bias=bcs["bq1c"][:, mi:mi + 1])
                gemm_TN(w_d["wq1"], lambda kc, n: dc_own[:, kc, :], 8, 8, 1,
                        drain_q1, wp, ps)

                def drain_k1(mi, n, pa):
                    nc.scalar.activation(kTv[:, mi, n * 512:(n + 1) * 512], pa[:],
                                         AF.Identity, bias=bcs["bk1c"][:, mi:mi + 1])
                gemm_TN(w_d["wk1"], lambda kc, n: dke[:, kc, n * 512:(n + 1) * 512],
                        8, 8, 2, drain_k1, wp, ps)

                def drain_v1(rc, nf, pa):
                    dst = vaug[:, rc, nf * 8:(nf + 1) * 8, 0:64]
                    src = pa[:].rearrange("p (h d) -> p h d", h=8)
                    nc.scalar.activation(dst, src, AF.Copy, scale=vm1[:, rc:rc + 1])
                gemm_NT(w_d["wq1"], dke, 8, 8, 2, drain_v1, wp, ps)
                for rc in range(8):
                    nc.sync.dma_start(out=vaug[:, rc, :, 64:65],
                                      in_=vm1r_d[:, rc, :].bitcast(f32r))

            # ================= phase 2: self attention =================
            with tc.tile_pool(name="mk2", bufs=1) as mkp, \
                 tc.tile_pool(name="sp2", bufs=2) as spool, \
                 tc.tile_pool(name="ep2", bufs=4) as epool, \
                 tc.tile_pool(name="up2", bufs=2) as upool, \
                 tc.tile_pool(name="pss", bufs=2, space="PSUM") as ps_s, \
                 tc.tile_pool(name="psav", bufs=2, space="PSUM") as ps_av, \
                 tc.tile_pool(name="psb", bufs=2, space="PSUM") as ps_b:
                masks = mkp.tile([128, 4, R], f32, name="masks")
                for c in range(4):
                    nc.scalar.dma_start(out=masks[:, c, :],
                                        in_=mask_d[c * 128:(c + 1) * 128, :])
                attention(qbuf, kTv, vaug, abufA, True, masks, spool, epool,
                          upool, ps_s, ps_av, ps_b)

            # ================= phase 3: cross projections =================
            with tc.tile_pool(name="ps3", bufs=6, space="PSUM") as ps:
                wp = wall
                nc.scalar.dma_start(out=encv,
                                    in_=enc_d.rearrange("(f p) r -> p f r", p=128).bitcast(f32r))

                def drain_q2(mi, n, pa):
                    nc.scalar.activation(qbuf[:, mi, :], pa[:], AF.Identity,
                                         bias=bcs["bq2c"][:, mi:mi + 1])
                gemm_TN(w_d["wq2"], lambda kc, n: dc_own[:, kc, :], 8, 8, 1,
                        drain_q2, wp, ps)

                def drain_k2(mi, n, pa):
                    nc.scalar.activation(kT2v[:, mi, n * 512:(n + 1) * 512], pa[:],
                                         AF.Identity, bias=bcs["bk2c"][:, mi:mi + 1])
                gemm_TN(w_d["wk2"], lambda kc, n: encv[:, kc, n * 512:(n + 1) * 512],
                        8, 8, 2, drain_k2, wp, ps)

                def drain_v2(rc, nf, pa):
                    dst = vaug[:, rc, nf * 8:(nf + 1) * 8, 0:64]
                    src = pa[:].rearrange("p (h d) -> p h d", h=8)
                    nc.scalar.activation(dst, src, AF.Copy, scale=vm2[:, rc:rc + 1])
                gemm_NT(w_d["wq2"], encv, 8, 8, 2, drain_v2, wp, ps)
                for rc in range(8):
                    nc.sync.dma_start(out=vaug[:, rc, :, 64:65],
                                      in_=vm2r_d[:, rc, :].bitcast(f32r))

            # ================= phase 4: cross attention =================
            with tc.tile_pool(name="sp4", bufs=2) as spool, \
                 tc.tile_pool(name="ep4", bufs=4) as epool, \
                 tc.tile_pool(name="up4", bufs=2) as upool, \
                 tc.tile_pool(name="pss4", bufs=2, space="PSUM") as ps_s, \
                 tc.tile_pool(name="psav4", bufs=2, space="PSUM") as ps_av, \
                 tc.tile_pool(name="psb4", bufs=2, space="PSUM") as ps_b:
                attention(qbuf, kT2v, vaug, abufB, False, None, spool, epool,
                          upool, ps_s, ps_av, ps_b)

            # ============ phase 5: output projections + LN1/LN2 ============
            with tc.tile_pool(name="tw5", bufs=2) as twp, \
                 tc.tile_pool(name="sq5", bufs=2) as sqp, \
                 tc.tile_pool(name="lt5", bufs=1) as ltp, \
                 tc.tile_pool(name="ps5", bufs=4, space="PSUM") as ps, \
                 tc.tile_pool(name="ps5ln", bufs=1, space="PSUM") as ps_ln:
                wp = wall

                def drain_wo1(mi, n, pa):
                    tw = twp.tile([128, 512], f32, name="tw")
                    nc.scalar.activation(tw[:], pa[:], AF.Identity,
                                         bias=bcs["bo1c"][:, mi:mi + 1])
                    with nc.allow_low_precision(reason="f32r keeps fp32 bits"):
                        nc.vector.tensor_add(xa[:, mi, :], tw[:], dc_own[:, mi, :])
                gemm_TN(w_d["wo1"], lambda kc, n: abufA[:, kc, :], 8, 8, 1,
                        drain_wo1, wp, ps)

                layernorm(xa, bcs["g1c"], bcs["b1c"], xa, sqp, ltp, ps_ln)

                def drain_wo2(mi, n, pa):
                    tw = twp.tile([128, 512], f32, name="tw")
                    nc.scalar.activation(tw[:], pa[:], AF.Identity,
                                         bias=bcs["bo2c"][:, mi:mi + 1])
                    with nc.allow_low_precision(reason="f32r keeps fp32 bits"):
                        nc.vector.tensor_add(abufA[:, mi, :], tw[:], xa[:, mi, :])
                gemm_TN(w_d["wo2"], lambda kc, n: abufB[:, kc, :], 8, 8, 1,
                        drain_wo2, wp, ps)

                layernorm(abufA, bcs["g2c"], bcs["b2c"], abufA, sqp, ltp, ps_ln)

            # ================= phase 6: FFN + LN3 + output =================
            with tc.tile_pool(name="tw6", bufs=2) as twp, \
                 tc.tile_pool(name="ps6", bufs=4, space="PSUM") as ps, \
                 tc.tile_pool(name="ps6b", bufs=1, space="PSUM") as ps8:
                wp = wall

                def drain_f1(mi, n, pa):
                    nc.scalar.activation(arena[:, mi * 512:(mi + 1) * 512], pa[:],
                                         AF.Relu, bias=fb1c[:, mi:mi + 1])
                gemm_TN(fw1_d, lambda kc, n: abufA[:, kc, :], 8, 32, 1,
                        drain_f1, wp, ps)

                # ffn2 in two 4-bank halves so it shares PSUM with ffn1 and
                # its matmuls can fill ffn1's weight-DMA gaps.
                for mh in range(2):
                    pps = [ps8.tile([128, 512], f32, name=f"pf_{i}")
                           for i in range(4)]
                    for kc in range(32):
                        wt = wp.tile([128, 512], f32r, name="wt")
                        nc.scalar.dma_start(
                            out=wt,
                            in_=fw2_d[kc * 128:(kc + 1) * 128,
                                      mh * 512:(mh + 1) * 512].bitcast(f32r))
                        for i in range(4):
                            nc.tensor.matmul(
                                pps[i][:],
                                lhsT=wt[:, i * 128:(i + 1) * 128],
                                rhs=arena[:, kc * 512:(kc + 1) * 512],
                                start=(kc == 0), stop=(kc == 31))
                    for i in range(4):
                        mi = mh * 4 + i
                        tw = twp.tile([128, 512], f32, name="tw")
                        nc.scalar.activation(tw[:], pps[i][:], AF.Identity,
                                             bias=bcs["fb2c"][:, mi:mi + 1])
                        with nc.allow_low_precision(reason="f32r keeps fp32 bits"):
                            nc.vector.tensor_add(xa[:, mi, :], tw[:], abufA[:, mi, :])

            with tc.tile_pool(name="sq7", bufs=2) as sqp, \
                 tc.tile_pool(name="lt7", bufs=1) as ltp, \
                 tc.tile_pool(name="ps7ln", bufs=1, space="PSUM") as ps_ln:
                layernorm(xa, bcs["g3c"], bcs["b3c"], qbuf, sqp, ltp, ps_ln)
                for mi in range(8):
                    nc.sync.dma_start(out=out_d[mi * 128:(mi + 1) * 128, :].bitcast(f32r),
                                      in_=qbuf[:, mi, :])
            ctx_wp.__exit__(None, None, None)

    _split_waits(nc, 1)
    return nc


_PROGRAM = None


def _get_program():
    global _PROGRAM
    if _PROGRAM is None:
        _PROGRAM = build_program()
    return _PROGRAM


def _core_inputs(inp, c):
    b, j = c // 2, c % 2
    dec = np.asarray(inp["dec_input"][b], np.float32)      # [S, D]
    enc = np.asarray(inp["enc_output"][b], np.float32)
    decT = np.ascontiguousarray(dec.T)                     # [D, S]
    own = np.ascontiguousarray(decT[:, j * R:(j + 1) * R])
    if j == 1:
        dke = decT                                         # ctx = rows 0:512, diag = 512:1024
    else:
        dke = np.ascontiguousarray(
            np.concatenate([decT[:, R:], decT[:, :R]], axis=1))
    la = np.asarray(inp["look_ahead_mask"], np.float32)[0, 0]
    maskT = np.ascontiguousarray(la[j * R:(j + 1) * R, j * R:(j + 1) * R].T) * np.float32(-8e9)
    padb = (np.asarray(inp["padding_mask"], np.float32)[b, 0, 0] * np.float32(-1e9))
    vm = np.ones(S, np.float32)
    if j == 0:
        vm[:R] = 0.0                                       # ctx block invalid for first half
    v2 = np.ones(S, np.float32)

    def chunk(a, n):
        return np.ascontiguousarray(np.asarray(a, np.float32).reshape(n, 128).T)

    wo1 = np.asarray(inp["wo1"], np.float32)
    wo2 = np.asarray(inp["wo2"], np.float32)
    bo1e = np.asarray(inp["bq1"], np.float32) @ wo1 + np.asarray(inp["bo1"], np.float32)
    bo2e = np.asarray(inp["bq2"], np.float32) @ wo2 + np.asarray(inp["bo2"], np.float32)

    return {
        "dc_own": own, "dke": dke,
        "encT": np.ascontiguousarray(enc.T),
        "maskT": maskT,
        "padb": chunk(padb, 8),
        "vm1": chunk(vm, 8),
        "vm1r": np.repeat(chunk(vm, 8)[:, :, None], 16, axis=2),
        "vm2": chunk(v2, 8),
        "vm2r": np.ones((128, 8, 16), np.float32),
        "onesd": np.ones((128, 128), np.float32),
        "wq1": np.asarray(inp["wq1"], np.float32),
        "wk1": np.asarray(inp["wk1"], np.float32),
        "wq2": np.asarray(inp["wq2"], np.float32),
        "wk2": np.asarray(inp["wk2"], np.float32),
        "wo1": wo1, "wo2": wo2,
        "fw1": np.asarray(inp["ff_w1"], np.float32),
        "fw2": np.asarray(inp["ff_w2"], np.float32),
        "bq1c": chunk(inp["bq1"], 8), "bk1c": chunk(inp["bk1"], 8),
        "bq2c": chunk(inp["bq2"], 8), "bk2c": chunk(inp["bk2"], 8),
        "bo1c": chunk(bo1e, 8), "bo2c": chunk(bo2e, 8),
        "fb1c": chunk(inp["ff_b1"], 32), "fb2c": chunk(inp["ff_b2"], 8),
        "g1c": chunk(inp["ln1_g"], 8), "b1c": chunk(inp["ln1_b"], 8),
        "g2c": chunk(inp["ln2_g"], 8), "b2c": chunk(inp["ln2_b"], 8),
        "g3c": chunk(inp["ln3_g"], 8), "b3c": chunk(inp["ln3_b"], 8),
    }


def kernel(**inputs):
    nc = _get_program()
    in_maps = [_core_inputs(inputs, c) for c in range(N_CORES)]
    res = run_bass_kernel_spmd(nc, in_maps, list(range(N_CORES)))
    out = np.empty((B, S, D), np.float32)
    for c in range(N_CORES):
        b, j = c // 2, c % 2
        out[b, j * R:(j + 1) * R, :] = res.results[c]["outT"].T
    return out


if __name__ == "__main__":
    import tempfile
    from concourse.bass_utils import compile_bass_kernel
    nc = build_program()
    with tempfile.TemporaryDirectory() as td:
        compile_bass_kernel(nc, td)
    print("COMPILE OK")



# revision 11
# speedup vs baseline: 1.0308x; 1.0308x over previous
"""Trainium2 Bass kernel for nn_Decoding_Layer (dense transformer decoder layer).

Sharding: 8 cores = 4 batches x 2 sequence-halves (512 query rows per core,
no collectives). Restructured from the phase-serial baseline into four dense
super-phases so the PE never idles long enough to re-throttle the HAM clock:

  S0: self projections q1/k1/v1                  (PE-dense gemms)
  S1: self-attention f-loop  ~interleaved~ cross projections q2/k2/v2
  S2: cross-attention f-loop ~interleaved~ wo1 gemm + softmax-1 normalize
  S3: LN1 || wo2 -> LN2 -> FFN1 -> FFN2 -> LN3   (gemm-dense, fused LN stats)

Weights / keys / q / exp-scores are bf16 (halves weight DMA + LDWEIGHTS and
keeps narrow matmuls at 1 cycle/column); activations stay f32r.  Softmax
denominators are batched into one [16,512] DVE reciprocal per attention phase
(instead of 16 x 3.3us single-partition reciprocals) and broadcast back per
head with a one-hot selector matmul.  Causal diagonal blocks are narrowed to
their valid query range.  Dummy matmuls bridge the few unavoidable PE gaps so
the HAM stays at K=8/8.
"""

import sys

if "/opt/trn_rl_repo" not in sys.path:
    sys.path.insert(0, "/opt/trn_rl_repo")

import numpy as np
import ml_dtypes

import concourse.bass as bass
import concourse.mybir as mybir
import concourse.tile as tile
from concourse import bass_utils
from concourse.bass_utils import run_bass_kernel_spmd

# walrus ships with --enable-ldw-opt=false; enabling it lets codegen overlap
# per-matmul weight loads with the matmul stream.
_orig_run_command = bass_utils.run_command

def _patched_run_command(argv, **kw):
    return _orig_run_command(argv, **kw)

f32 = mybir.dt.float32
f32r = mybir.dt.float32r
bf16 = mybir.dt.bfloat16
AF = mybir.ActivationFunctionType
ALU = mybir.AluOpType
BF = ml_dtypes.bfloat16

B, S, D, H, DFF = 4, 1024, 1024, 16, 4096
R = 512
EPS = 1e-6
N_CORES = 8

# arena regions, bf16-element offsets per partition
_A = 0          # dc_own bf16 [128,8,512]         (S0..S2)
_B = 4096       # dke bf16 (S0)      -> kT2 bf16  (S1..S2)
_C = 12288      # encv bf16 (S0..S1) -> hidden.lo (S3)
_D = 20480      # kT1 bf16 (S0..S1)  -> hidden.hi (S3)
_E = 28672      # vaug1 bf16 (S0..S1)-> abufB bf16(S2..S3)
_F = 36992      # vaug2 bf16 (S1..S2)-> x2 bf16   (S3)
_G = 45312      # qbuf1 bf16 (S0..S1)
_H = 49408      # qbuf2 bf16 (S1..S2)
_I = 53504      # abufA bf16 (S1..S2)
_J = 57600      # xa/x1 f32r (S2..S3) -> x3/out   (S3)
_TOT = 65792


def _split_waits(nc, maxw=1):
    """Walrus encodes at most one semaphore wait per instruction; move excess
    waits onto same-engine NOPs placed immediately before."""
    for f in nc.m.functions:
        for bb in f.blocks:
            out = []
            for inst in bb.instructions:
                si = inst.sync_info
                if si is not None and len(si.on_wait) > maxw:
                    waits = list(si.on_wait)
                    keep, excess = waits[-maxw:], waits[:-maxw]
                    eng = getattr(inst, "engine", None)
                    k = 0
                    while excess:
                        chunk, excess = excess[:maxw], excess[maxw:]
                        out.append(mybir.InstNoOp(
                            name=f"{inst.name}_wsp{k}",
                            engine=eng,
                            bass_nofuse=True,
                            sync_info=mybir.SyncInfo(on_wait=chunk, on_update=[]),
                        ))
                        k += 1
                    inst.sync_info = mybir.SyncInfo(
                        on_wait=keep, on_update=list(si.on_update))
                out.append(inst)
            bb.instructions = out


def _pull(gens, n):
    done = 0
    while gens and done < n:
        try:
            next(gens[0])
            done += 1
        except StopIteration:
            gens.pop(0)


def _exhaust(gens):
    while gens:
        try:
            next(gens[0])
        except StopIteration:
            gens.pop(0)


def build_program():
    nc = bass.Bass("TRN2", target_bir_lowering=False, debug=False)

    def din(name, shape, dt=f32):
        return nc.dram_tensor(name, shape, dt, kind="ExternalInput").ap()

    dc_own_d = din("dc_own", [D, R], bf16)
    dke_d = din("dke", [D, S], bf16)
    enc_d = din("encT", [D, S], bf16)
    mask_d = din("mask128", [128, 128])          # diag causal block [k,q] * -8e9
    padb_d = din("padb", [128, 8])               # -1e9 * padding_mask, chunked
    vm1_d = din("vm1", [128, 8])                 # self V-row mask (drain scale)
    vm1r_d = din("vm1r", [128, 8, 16], bf16)     # ones-row, self
    vm2r_d = din("vm2r", [128, 8, 16], bf16)     # ones-row, cross
    ones_d = din("onesb", [128, 128], bf16)
    sel_d = din("sel16", [16, 1024])             # one-hot head selector
    onesf_d = din("onesf", [128, 128])           # f32 ones
    w_d = {k: din(k, [D, D], bf16)
           for k in ("wq1", "wk1", "wq2", "wk2", "wo1", "wo2")}
    fw1_d = din("fw1", [D, DFF], bf16)
    fw2_d = din("fw2", [DFF, D], bf16)
    bc_d = {k: din(k, [128, 8]) for k in
            ("bq1c", "bk1c", "bq2c", "bk2c", "bo1c", "bo2c", "fb2c",
             "g1c", "b1c", "g2c", "b2c", "g3c", "b3c")}
    fb1c_d = din("fb1c", [128, 32])
    out_d = nc.dram_tensor("outT", [D, R], f32, kind="ExternalOutput").ap()

    with tile.TileContext(nc) as tc:
        with tc.tile_pool(name="persist", bufs=1) as pp, \
             tc.tile_pool(name="consts", bufs=1) as cp:
            arena = pp.tile([128, _TOT], bf16, name="arena")

            def reg_bf(o, n, f):
                return arena[:, o:o + n].rearrange("p (f r) -> p f r", f=f)

            def reg_fr(o, n, f):
                return arena[:, o:o + n].bitcast(f32r).rearrange(
                    "p (f r) -> p f r", f=f)

            dc_own = reg_bf(_A, 4096, 8)             # [128,8,512] bf16
            dke = reg_bf(_B, 8192, 8)                # [128,8,1024] bf16
            kT2 = reg_bf(_B, 8192, 8)
            encv = reg_bf(_C, 8192, 8)
            kT1 = reg_bf(_D, 8192, 8)
            hidden = reg_bf(_C, 16384, 32)           # [128,32,512] bf16
            vaug1 = arena[:, _E:_E + 8320].rearrange(
                "p (f h v) -> p f h v", f=8, h=16)   # [128,8,16,65] bf16
            abufB = reg_bf(_E, 4096, 8)
            vaug2 = arena[:, _F:_F + 8320].rearrange(
                "p (f h v) -> p f h v", f=8, h=16)
            x2 = reg_bf(_F, 4096, 8)
            qbuf1 = reg_bf(_G, 4096, 8)              # [128,8,512] bf16
            qbuf2 = reg_bf(_H, 4096, 8)
            abufA = reg_bf(_I, 4096, 8)
            xat = pp.tile([128, 4096], f32r, name="xat")
            xa = xat.rearrange("p (f r) -> p f r", f=8)

            # ---- constants / small persistents ----
            onesb = cp.tile([128, 128], bf16, name="onesb")
            sel16 = cp.tile([16, 1024], f32r, name="sel16")
            onesbr = cp.tile([128, 128], f32r, name="onesbr")
            mask128 = cp.tile([128, 128], f32, name="mask128")
            padb = cp.tile([128, 8], f32, name="padb")
            vm1 = cp.tile([128, 8], f32, name="vm1")
            bcs = {k: cp.tile([128, 8], f32, name=k) for k in bc_d}
            fb1c = cp.tile([128, 32], f32, name="fb1c")
            den16 = cp.tile([16, 512], f32, name="den16")
            rec16 = cp.tile([16, 512], f32r, name="rec16")

            # startup DMAs: dc_own chunk0 first (warmup+q1), weights stream on
            # sync; dke on scalar; everything else on gpsimd.
            dco_r = dc_own_d.rearrange("(f p) r -> p f r", p=128)
            for kc in range(8):
                nc.sync.dma_start(out=dc_own[:, kc, :], in_=dco_r[:, kc, :])
            dke_r = dke_d.rearrange("(f p) r -> p f r", p=128)
            for kc in range(8):
                nc.scalar.dma_start(out=dke[:, kc, :], in_=dke_r[:, kc, :])
            nc.gpsimd.dma_start(out=bcs["bq1c"], in_=bc_d["bq1c"])
            nc.gpsimd.dma_start(out=bcs["bk1c"], in_=bc_d["bk1c"])
            nc.gpsimd.dma_start(out=onesb, in_=ones_d)
            nc.gpsimd.dma_start(out=sel16, in_=sel_d.bitcast(f32r))
            nc.gpsimd.dma_start(out=onesbr, in_=onesf_d.bitcast(f32r))
            nc.gpsimd.dma_start(out=mask128, in_=mask_d)
            nc.gpsimd.dma_start(out=padb, in_=padb_d)
            nc.gpsimd.dma_start(out=vm1, in_=vm1_d)
            for k in bcs:
                if k not in ("bq1c", "bk1c"):
                    nc.gpsimd.dma_start(out=bcs[k], in_=bc_d[k])
            nc.gpsimd.dma_start(out=fb1c, in_=fb1c_d)
            for rc in range(8):
                nc.gpsimd.dma_start(out=vaug1[:, rc, :, 64:65],
                                    in_=vm1r_d[:, rc, :])
            enc_r = enc_d.rearrange("(f p) r -> p f r", p=128)
            for kc in range(8):
                nc.gpsimd.dma_start(out=encv[:, kc, :], in_=enc_r[:, kc, :])

            ones1 = onesb[0:1, :]      # [1,128] bf16
            onesp = onesb[:, 0:1]      # [128,1] bf16
            ones1r = onesbr[0:1, :]    # [1,128] f32r
            onespr = onesbr[:, 0:1]    # [128,1] f32r

            wall = ctx_wp = tc.tile_pool(name="wall", bufs=8)
            wall = wall.__enter__()

            def lp():
                return nc.allow_low_precision(reason="f32r/bf16 by design")

            # ---------------- generator gemm helpers ----------------
            def gemm_TN(Wd, xt, KCn, MCn, NN, drain, ps, dq):
                """OUT^T[mi, n] = sum_kc W[kc,mi]^T @ xt(kc,n). one mi/group."""
                for mi in range(MCn):
                    pps = [ps.tile([128, 512], f32, name="pp")
                           for _ in range(NN)]
                    for kc in range(KCn):
                        wt = wall.tile([128, 128], bf16, name="wt")
                        dq(out=wt, in_=Wd[kc * 128:(kc + 1) * 128,
                                          mi * 128:(mi + 1) * 128])
                        for n in range(NN):
                            nc.tensor.matmul(pps[n][:], lhsT=wt[:],
                                             rhs=xt(kc, n),
                                             start=(kc == 0),
                                             stop=(kc == KCn - 1))
                        yield
                    for n in range(NN):
                        drain(mi, n, pps[n])
                    yield

            def gemm_NT(Wd, xt_sb, KCn, RCn, NFn, g, drain, ps, dq):
                """OUT[rc] = X @ W; lhsT = xT chunks, rhs = W col tiles."""
                for nf in range(NFn):
                    for rg in range(0, RCn, g):
                        gs = min(g, RCn - rg)
                        pps = [ps.tile([128, 512], f32, name="pp")
                               for _ in range(gs)]
                        for kc in range(KCn):
                            wt = wall.tile([128, 512], bf16, name="wtn")
                            dq(out=wt, in_=Wd[kc * 128:(kc + 1) * 128,
                                              nf * 512:(nf + 1) * 512])
                            for i in range(gs):
                                nc.tensor.matmul(
                                    pps[i][:],
                                    lhsT=xt_sb[:, kc,
                                               (rg + i) * 128:(rg + i + 1) * 128],
                                    rhs=wt[:],
                                    start=(kc == 0), stop=(kc == KCn - 1))
                            yield
                        for i in range(gs):
                            drain(rg + i, nf, pps[i])
                        yield

            # ---------------- attention phase ----------------
            def attention(q_sb, kT, va, out_sb, is_self, pulls,
                          ssp, avp, ep, stp):
                kc_order = [0, 1, 2, 3, 7, 6, 5, 4] if is_self else range(8)
                kc_order = list(kc_order)
                for f in range(8):
                    avs = [avp.tile([65, 512], f32, name="av")
                           for _ in range(2)]
                    prev = None
                    for idx, kc in enumerate(kc_order):
                        c0 = (kc - 4) * 128 if (is_self and kc >= 4) else 0
                        sss = [ssp.tile([128, 512], f32, name="ss")
                               for _ in range(2)]
                        for a in range(2):
                            nc.tensor.matmul(
                                sss[a][:, c0:512],
                                lhsT=kT[64 * a:64 * (a + 1), f,
                                        kc * 128:(kc + 1) * 128],
                                rhs=q_sb[64 * a:64 * (a + 1), f, c0:512],
                                start=True, stop=True)
                        if prev is not None:
                            pes, pc0, pkc, pidx = prev
                            for a in range(2):
                                nc.tensor.matmul(
                                    avs[a][:, pc0:512],
                                    lhsT=va[:, pkc, 2 * f + a, :],
                                    rhs=pes[a][:, pc0:512],
                                    start=(pidx == 0), stop=False)
                        if is_self and kc >= 4:
                            for a in range(2):
                                nc.vector.tensor_tensor(
                                    out=sss[a][:, c0:c0 + 128],
                                    in0=sss[a][:, c0:c0 + 128],
                                    in1=mask128[:], op=ALU.add)
                        es = [ep.tile([128, 512], bf16, name="ee")
                              for _ in range(2)]
                        for a in range(2):
                            bias = 0.0 if is_self else padb[:, kc:kc + 1]
                            nc.scalar.activation(es[a][:, c0:512],
                                                 sss[a][:, c0:512],
                                                 AF.Exp, bias=bias, scale=0.125)
                        prev = (es, c0, kc, idx)
                        _pull(pulls, 3)
                    pes, pc0, pkc, pidx = prev
                    for a in range(2):
                        nc.tensor.matmul(avs[a][:, pc0:512],
                                         lhsT=va[:, pkc, 2 * f + a, :],
                                         rhs=pes[a][:, pc0:512],
                                         start=False, stop=True)
                    for a in range(2):
                        h = 2 * f + a
                        stg = stp.tile([1, 512], f32, name="stg")
                        with lp():
                            nc.vector.tensor_scalar_mul(
                                out_sb[64 * a:64 * (a + 1), f, :],
                                avs[a][0:64, :], 1.0)
                            nc.vector.tensor_scalar_mul(
                                stg[:], avs[a][64:65, :], 1.0)
                        nc.gpsimd.dma_start(out=den16[h:h + 1, :], in_=stg[:])
                    _pull(pulls, 2)

            def phase_end(out_sb, bpp):
                with lp():
                    nc.vector.reciprocal(rec16[:], den16[:])
                yield
                for f in range(8):
                    for a in range(2):
                        h = 2 * f + a
                        bpt = bpp.tile([64, 512], f32, name="bp")
                        nc.tensor.matmul(bpt[:],
                                         lhsT=sel16[:, h * 64:(h + 1) * 64],
                                         rhs=rec16[:], start=True, stop=True)
                        with lp():
                            nc.vector.tensor_mul(
                                out_sb[64 * a:64 * (a + 1), f, :],
                                out_sb[64 * a:64 * (a + 1), f, :], bpt[:])
                        yield

            # ---------------- layernorm helpers ----------------
            def ln_stat(x_chunk, xones, pm, pv, first, last, sqp):
                nc.tensor.matmul(pm[:], lhsT=xones, rhs=x_chunk,
                                 start=first, stop=last)
                sq = sqp.tile([128, 512], f32r, name="sq")
                nc.scalar.activation(sq[:], x_chunk, AF.Square)
                nc.tensor.matmul(pv[:], lhsT=onespr, rhs=sq[:],
                                 start=first, stop=last)

            def ln_finalize(x_sb, pm, pv, gC, bC, ltp, bcp):
                """normalize x_sb [128,8,512] in place; yields between steps."""
                m = ltp.tile([1, 512], f32, name="lm")
                sc = ltp.tile([1, 512], f32, name="lsc")
                sc2 = ltp.tile([1, 512], f32, name="lsc2")
                inv = ltp.tile([1, 512], f32r, name="linv")
                minv = ltp.tile([1, 512], f32r, name="lminv")
                nc.vector.tensor_scalar_mul(m[:], pm[:], 1.0 / D)
                nc.vector.tensor_scalar_mul(sc[:], pv[:], 1.0 / D)
                nc.vector.tensor_mul(sc2[:], m[:], m[:])
                nc.vector.tensor_scalar_add(sc2[:], sc2[:], -EPS)
                nc.vector.tensor_tensor(out=sc[:], in0=sc[:], in1=sc2[:],
                                        op=ALU.subtract)
                nc.scalar.activation(sc[:], sc[:], AF.Sqrt)
                with lp():
                    nc.vector.reciprocal(inv[:], sc[:])
                    nc.vector.tensor_mul(minv[:], m[:], inv[:])
                yield
                binv = bcp.tile([128, 512], f32, name="binv")
                bmv = bcp.tile([128, 512], f32, name="bmv")
                nc.tensor.matmul(binv[:], lhsT=ones1r, rhs=inv[:],
                                 start=True, stop=True)
                nc.tensor.matmul(bmv[:], lhsT=ones1r, rhs=minv[:],
                                 start=True, stop=True)
                yield
                for c in range(8):
                    with lp():
                        nc.vector.tensor_tensor(out=x_sb[:, c, :],
                                                in0=x_sb[:, c, :],
                                                in1=binv[:], op=ALU.mult)
                        nc.vector.tensor_tensor(out=x_sb[:, c, :],
                                                in0=x_sb[:, c, :],
                                                in1=bmv[:], op=ALU.subtract)
                    nc.scalar.activation(x_sb[:, c, :], x_sb[:, c, :],
                                         AF.Identity, bias=bC[:, c:c + 1],
                                         scale=gC[:, c:c + 1])
                    yield

            def dummy_fill(ps, n):
                for i in range(n):
                    dt = ps.tile([128, 512], f32, name="pp")
                    nc.tensor.matmul(dt[:], lhsT=onesb[:, 0:128],
                                     rhs=dc_own[:, 0, :], start=True, stop=True)

            # ================= S0: warmup + self projections =================
            with tc.tile_pool(name="warmp", bufs=1, space="PSUM") as wps, \
                 tc.tile_pool(name="ps0", bufs=4, space="PSUM") as ps0:
                wtile = wps.tile([128, 512], f32, name="warm")
                for wi in range(24):
                    nc.tensor.matmul(wtile[:], lhsT=onesb[:, 0:128],
                                     rhs=dc_own[:, 0, :],
                                     start=(wi == 0), stop=(wi == 23))

                def drain_q1(mi, n, pa):
                    nc.scalar.activation(qbuf1[:, mi, :], pa[:], AF.Identity,
                                         bias=bcs["bq1c"][:, mi:mi + 1])
                g = gemm_TN(w_d["wq1"], lambda kc, n: dc_own[:, kc, :],
                            8, 8, 1, drain_q1, ps0, nc.sync.dma_start)
                _exhaust([g])

                def drain_k1(mi, n, pa):
                    nc.scalar.activation(kT1[:, mi, n * 512:(n + 1) * 512],
                                         pa[:], AF.Identity,
                                         bias=bcs["bk1c"][:, mi:mi + 1])
                g = gemm_TN(w_d["wk1"],
                            lambda kc, n: dke[:, kc, n * 512:(n + 1) * 512],
                            8, 8, 2, drain_k1, ps0, nc.sync.dma_start)
                _exhaust([g])

                def drain_v1(rc, nf, pa):
                    dst = vaug1[:, rc, nf * 8:(nf + 1) * 8, 0:64]
                    src = pa[:].rearrange("p (h d) -> p h d", h=8)
                    nc.scalar.activation(dst, src, AF.Copy,
                                         scale=vm1[:, rc:rc + 1])
                g = gemm_NT(w_d["wq1"], dke, 8, 8, 2, 4, drain_v1,
                            ps0, nc.sync.dma_start)
                _exhaust([g])

            # vaug2 ones-rows can land any time before attn2
            for rc in range(8):
                nc.gpsimd.dma_start(out=vaug2[:, rc, :, 64:65],
                                    in_=vm2r_d[:, rc, :])

            # ============ S1 + S2: attention with interleaved gemms ============
            with tc.tile_pool(name="ssp", bufs=2, space="PSUM") as ssp, \
                 tc.tile_pool(name="avp", bufs=2, space="PSUM") as avp, \
                 tc.tile_pool(name="bppA", bufs=2, space="PSUM") as bppA, \
                 tc.tile_pool(name="gemA", bufs=2, space="PSUM") as gemA, \
                 tc.tile_pool(name="ep", bufs=4) as ep, \
                 tc.tile_pool(name="stp", bufs=3) as stp:

                # S1: self attention + cross projections
                def drain_q2(mi, n, pa):
                    nc.scalar.activation(qbuf2[:, mi, :], pa[:], AF.Identity,
                                         bias=bcs["bq2c"][:, mi:mi + 1])

                def drain_k2(mi, n, pa):
                    nc.scalar.activation(kT2[:, mi, n * 512:(n + 1) * 512],
                                         pa[:], AF.Identity,
                                         bias=bcs["bk2c"][:, mi:mi + 1])

                def drain_v2(rc, nf, pa):
                    dst = vaug2[:, rc, nf * 8:(nf + 1) * 8, 0:64]
                    src = pa[:].rearrange("p (h d) -> p h d", h=8)
                    nc.scalar.activation(dst, src, AF.Copy)

                s1_gens = [
                    gemm_TN(w_d["wq2"], lambda kc, n: dc_own[:, kc, :],
                            8, 8, 1, drain_q2, gemA, nc.sync.dma_start),
                    gemm_TN(w_d["wk2"],
                            lambda kc, n: encv[:, kc, n * 512:(n + 1) * 512],
                            8, 8, 2, drain_k2, gemA, nc.sync.dma_start),
                    gemm_NT(w_d["wq2"], encv, 8, 8, 2, 2, drain_v2,
                            gemA, nc.sync.dma_start),
                ]
                attention(qbuf1, kT1, vaug1, abufA, True, s1_gens,
                          ssp, avp, ep, stp)
                _exhaust(s1_gens)

                # S2: cross attention + softmax1 normalize + wo1
                def drain_wo1(mi, n, pa):
                    tw = twp.tile([128, 512], f32, name="tw")
                    nc.scalar.activation(tw[:], pa[:], AF.Identity,
                                         bias=bcs["bo1c"][:, mi:mi + 1])
                    with lp():
                        nc.vector.tensor_add(xa[:, mi, :], tw[:],
                                             dc_own[:, mi, :])

                with tc.tile_pool(name="twp", bufs=2) as twp:
                    s2_gens = [
                        phase_end(abufA, bppA),
                        gemm_TN(w_d["wo1"], lambda kc, n: abufA[:, kc, :],
                                8, 8, 1, drain_wo1, gemA, nc.sync.dma_start),
                    ]
                    attention(qbuf2, kT2, vaug2, abufB, False, s2_gens,
                              ssp, avp, ep, stp)
                    _exhaust(s2_gens)

            # ================= S3a: LN1 || wo2 -> LN2 =================
            with tc.tile_pool(name="bppB", bufs=2, space="PSUM") as bppB, \
                 tc.tile_pool(name="gemB", bufs=2, space="PSUM") as gemB, \
                 tc.tile_pool(name="lnP", bufs=1, space="PSUM") as lnP, \
                 tc.tile_pool(name="bcP", bufs=1, space="PSUM") as bcP, \
                 tc.tile_pool(name="sqp", bufs=2) as sqp, \
                 tc.tile_pool(name="ltp", bufs=1) as ltp, \
                 tc.tile_pool(name="twp2", bufs=2) as twp2:
                # LN1 stats on xa (PE work covering the softmax2 reciprocal)
                pm1 = lnP.tile([1, 512], f32, name="pm")
                pv1 = lnP.tile([1, 512], f32, name="pv")
                for c in range(8):
                    ln_stat(xa[:, c, :], onespr, pm1, pv1, c == 0, c == 7, sqp)
                # softmax2 normalize (recip on DVE; bp matmuls on PE)
                _exhaust([phase_end(abufB, bppB)])
                # LN1 finalize fully (its final ACTs must precede wo2 drains)
                ln1 = ln_finalize(xa, pm1, pv1, bcs["g1c"], bcs["b1c"],
                                  ltp, bcP)
                _exhaust([ln1])
                dummy_fill(gemB, 16)

                # wo2 gemm; drains add x1 residual and feed LN2 stats
                pm2 = lnP.tile([1, 512], f32, name="pm")
                pv2 = lnP.tile([1, 512], f32, name="pv")

                def drain_wo2(mi, n, pa):
                    tw = twp2.tile([128, 512], f32, name="tw2")
                    nc.scalar.activation(tw[:], pa[:], AF.Identity,
                                         bias=bcs["bo2c"][:, mi:mi + 1])
                    with lp():
                        nc.vector.tensor_add(x2[:, mi, :], tw[:], xa[:, mi, :])
                    ln_stat(x2[:, mi, :], onesp, pm2, pv2, mi == 0, mi == 7, sqp)

                _exhaust([gemm_TN(w_d["wo2"], lambda kc, n: abufB[:, kc, :],
                                  8, 8, 1, drain_wo2, gemB,
                                  nc.sync.dma_start)])
                dummy_fill(gemB, 20)
                _exhaust([ln_finalize(x2, pm2, pv2, bcs["g2c"], bcs["b2c"],
                                      ltp, bcP)])

            # ================= S3b: FFN + LN3 + output =================
            with tc.tile_pool(name="gemC", bufs=2, space="PSUM") as gemC, \
                 tc.tile_pool(name="ffnP", bufs=2, space="PSUM") as ffnP, \
                 tc.tile_pool(name="lnP2", bufs=1, space="PSUM") as lnP2, \
                 tc.tile_pool(name="bcP2", bufs=1, space="PSUM") as bcP2, \
                 tc.tile_pool(name="sqp2", bufs=2) as sqp2, \
                 tc.tile_pool(name="ltp2", bufs=1) as ltp2, \
                 tc.tile_pool(name="twp3", bufs=2) as twp3:

                def drain_f1(mi, n, pa):
                    nc.scalar.activation(hidden[:, mi, :], pa[:], AF.Relu,
                                         bias=fb1c[:, mi:mi + 1])
                _exhaust([gemm_TN(fw1_d, lambda kc, n: x2[:, kc, :],
                                  8, 32, 1, drain_f1, gemC,
                                  nc.scalar.dma_start)])

                pm3 = lnP2.tile([1, 512], f32, name="pm3")
                pv3 = lnP2.tile([1, 512], f32, name="pv3")
                # ffn2 in quarters (2 out-chunks each) so PSUM stays at 2 banks
                for qd in range(4):
                    pps = [ffnP.tile([128, 512], f32, name="pf")
                           for _ in range(2)]
                    for kc in range(32):
                        wt = wall.tile([128, 256], bf16, name="wtf")
                        dq = nc.sync.dma_start if kc % 2 == 0 \
                            else nc.gpsimd.dma_start
                        dq(out=wt, in_=fw2_d[kc * 128:(kc + 1) * 128,
                                             qd * 256:(qd + 1) * 256])
                        for i in range(2):
                            nc.tensor.matmul(
                                pps[i][:], lhsT=wt[:, i * 128:(i + 1) * 128],
                                rhs=hidden[:, kc, :],
                                start=(kc == 0), stop=(kc == 31))
                    for i in range(2):
                        mi = qd * 2 + i
                        tw = twp3.tile([128, 512], f32, name="tw3")
                        nc.scalar.activation(tw[:], pps[i][:], AF.Identity,
                                             bias=bcs["fb2c"][:, mi:mi + 1])
                        with lp():
                            nc.vector.tensor_add(xa[:, mi, :], tw[:],
                                                 x2[:, mi, :])
                        ln_stat(xa[:, mi, :], onespr, pm3, pv3, mi == 0, mi == 7, sqp2)

                _exhaust([ln_finalize(xa, pm3, pv3, bcs["g3c"], bcs["b3c"],
                                      ltp2, bcP2)])
                for mi in range(8):
                    nc.sync.dma_start(
                        out=out_d[mi * 128:(mi + 1) * 128, :].bitcast(f32r),
                        in_=xa[:, mi, :])
            ctx_wp.__exit__(None, None, None)

    _split_waits(nc, 1)
    return nc


_PROGRAM = None


def _get_program():
    global _PROGRAM
    if _PROGRAM is None:
        _PROGRAM = build_program()
    return _PROGRAM


def _chunk(a, n):
    return np.ascontiguousarray(np.asarray(a, np.float32).reshape(n, 128).T)


def _shared_inputs(inp):
    """Per-run (not per-core) conversions."""
    sh = {}
    for k in ("wq1", "wk1", "wq2", "wk2", "wo1", "wo2"):
        sh[k] = np.ascontiguousarray(np.asarray(inp[k], np.float32).astype(BF))
    sh["fw1"] = np.ascontiguousarray(np.asarray(inp["ff_w1"], np.float32).astype(BF))
    sh["fw2"] = np.ascontiguousarray(np.asarray(inp["ff_w2"], np.float32).astype(BF))
    wo1 = np.asarray(inp["wo1"], np.float32)
    wo2 = np.asarray(inp["wo2"], np.float32)
    bo1e = np.asarray(inp["bq1"], np.float32) @ wo1 + np.asarray(inp["bo1"], np.float32)
    bo2e = np.asarray(inp["bq2"], np.float32) @ wo2 + np.asarray(inp["bo2"], np.float32)
    la = np.asarray(inp["look_ahead_mask"], np.float32)[0, 0]
    sh["mask128"] = np.ascontiguousarray(la[:128, :128].T) * np.float32(-8e9)
    sh["sel16"] = np.zeros((16, 1024), np.float32)
    for h in range(16):
        sh["sel16"][h, h * 64:(h + 1) * 64] = 1
    sh["onesb"] = np.ones((128, 128), BF)
    sh["onesf"] = np.ones((128, 128), np.float32)
    sh["vm2r"] = np.ones((128, 8, 16), BF)
    sh.update({
        "bq1c": _chunk(inp["bq1"], 8), "bk1c": _chunk(inp["bk1"], 8),
        "bq2c": _chunk(inp["bq2"], 8), "bk2c": _chunk(inp["bk2"], 8),
        "bo1c": _chunk(bo1e, 8), "bo2c": _chunk(bo2e, 8),
        "fb1c": _chunk(inp["ff_b1"], 32), "fb2c": _chunk(inp["ff_b2"], 8),
        "g1c": _chunk(inp["ln1_g"], 8), "b1c": _chunk(inp["ln1_b"], 8),
        "g2c": _chunk(inp["ln2_g"], 8), "b2c": _chunk(inp["ln2_b"], 8),
        "g3c": _chunk(inp["ln3_g"], 8), "b3c": _chunk(inp["ln3_b"], 8),
    })
    return sh


def _core_inputs(inp, sh, c):
    b, j = c // 2, c % 2
    dec = np.asarray(inp["dec_input"], np.float32)[b]        # [S, D]
    enc = np.asarray(inp["enc_output"], np.float32)[b]
    decT = np.ascontiguousarray(dec.T)                       # [D, S]
    own = np.ascontiguousarray(decT[:, j * R:(j + 1) * R])
    if j == 1:
        dkeT = decT                                          # ctx | own
    else:
        dkeT = np.concatenate([decT[:, R:], decT[:, :R]], axis=1)
    padb = (np.asarray(inp["padding_mask"], np.float32)[b, 0, 0]
            * np.float32(-1e9))
    vm = np.ones(S, np.float32)
    if j == 0:
        vm[:R] = 0.0
    vmc = _chunk(vm, 8)
    m = {
        "dc_own": np.ascontiguousarray(own.astype(BF)),
        "dke": np.ascontiguousarray(dkeT.astype(BF)),
        "encT": np.ascontiguousarray(enc.T.astype(BF)),
        "padb": _chunk(padb, 8),
        "vm1": vmc,
        "vm1r": np.ascontiguousarray(
            np.repeat(vmc[:, :, None], 16, axis=2).astype(BF)),
    }
    m.update(sh)
    return m


def kernel(**inputs):
    nc = _get_program()
    sh = _shared_inputs(inputs)
    in_maps = [_core_inputs(inputs, sh, c) for c in range(N_CORES)]
    res = run_bass_kernel_spmd(nc, in_maps, list(range(N_CORES)))
    out = np.empty((B, S, D), np.float32)
    for c in range(N_CORES):
        b, j = c // 2, c % 2
        out[b, j * R:(j + 1) * R, :] = res.results[c]["outT"].T
    return out


if __name__ == "__main__":
    import tempfile
    from concourse.bass_utils import compile_bass_kernel
    nc = build_program()
    with tempfile.TemporaryDirectory() as td:
        compile_bass_kernel(nc, td)
    print("COMPILE OK")


# revision 14
# speedup vs baseline: 1.3225x; 1.2830x over previous
"""Trainium2 Bass kernel for nn_Decoding_Layer (dense transformer decoder layer).

Sharding: 8 cores = 4 batches x 2 sequence-halves (512 query rows per core,
no collectives). Restructured from the phase-serial baseline into four dense
super-phases so the PE never idles long enough to re-throttle the HAM clock:

  S0: self projections q1/k1/v1                  (PE-dense gemms)
  S1: self-attention f-loop  ~interleaved~ cross projections q2/k2/v2
  S2: cross-attention f-loop ~interleaved~ wo1 gemm + softmax-1 normalize
  S3: LN1 || wo2 -> LN2 -> FFN1 -> FFN2 -> LN3   (gemm-dense, fused LN stats)

Weights / keys / q / exp-scores are bf16 (halves weight DMA + LDWEIGHTS and
keeps narrow matmuls at 1 cycle/column); activations stay f32r.  Softmax
denominators are batched into one [16,512] DVE reciprocal per attention phase
(instead of 16 x 3.3us single-partition reciprocals) and broadcast back per
head with a one-hot selector matmul.  Causal diagonal blocks are narrowed to
their valid query range.  Dummy matmuls bridge the few unavoidable PE gaps so
the HAM stays at K=8/8.
"""

import sys

if "/opt/trn_rl_repo" not in sys.path:
    sys.path.insert(0, "/opt/trn_rl_repo")

import numpy as np
import ml_dtypes

import concourse.bass as bass
import concourse.mybir as mybir
import concourse.tile as tile
from concourse import bass_utils
from concourse.bass_utils import run_bass_kernel_spmd

# walrus ships with --enable-ldw-opt=false; enabling it lets codegen overlap
# per-matmul weight loads with the matmul stream.
_orig_run_command = bass_utils.run_command

def _patched_run_command(argv, **kw):
    return _orig_run_command(argv, **kw)

f32 = mybir.dt.float32
f32r = mybir.dt.float32r
bf16 = mybir.dt.bfloat16
AF = mybir.ActivationFunctionType
ALU = mybir.AluOpType
BF = ml_dtypes.bfloat16

B, S, D, H, DFF = 4, 1024, 1024, 16, 4096
R = 512
EPS = 1e-6
N_CORES = 8

# arena regions, bf16-element offsets per partition
_A = 0          # dc_own bf16 [128,8,512]         (S0..S2)
_B = 4096       # dke bf16 (S0)      -> kT2 bf16  (S1..S2)
_C = 12288      # encv bf16 (S0..S1) -> hidden.lo (S3)
_D = 20480      # kT1 bf16 (S0..S1)  -> hidden.hi (S3)
_E = 28672      # vaug1 bf16 (S0..S1)-> abufB bf16(S2..S3)
_F = 36992      # vaug2 bf16 (S1..S2)-> x2 bf16   (S3)
_G = 45312      # qbuf1 bf16 (S0..S1)
_H = 49408      # qbuf2 bf16 (S1..S2)
_I = 53504      # abufA bf16 (S1..S2)
_J = 57600      # xa/x1 f32r (S2..S3) -> x3/out   (S3)
_TOT = 65792


def _split_waits(nc, maxw=1):
    """Walrus encodes at most one semaphore wait per instruction; move excess
    waits onto same-engine NOPs placed immediately before."""
    for f in nc.m.functions:
        for bb in f.blocks:
            out = []
            for inst in bb.instructions:
                si = inst.sync_info
                if si is not None and len(si.on_wait) > maxw:
                    waits = list(si.on_wait)
                    keep, excess = waits[-maxw:], waits[:-maxw]
                    eng = getattr(inst, "engine", None)
                    k = 0
                    while excess:
                        chunk, excess = excess[:maxw], excess[maxw:]
                        out.append(mybir.InstNoOp(
                            name=f"{inst.name}_wsp{k}",
                            engine=eng,
                            bass_nofuse=True,
                            sync_info=mybir.SyncInfo(on_wait=chunk, on_update=[]),
                        ))
                        k += 1
                    inst.sync_info = mybir.SyncInfo(
                        on_wait=keep, on_update=list(si.on_update))
                out.append(inst)
            bb.instructions = out


def _pull(gens, n):
    done = 0
    while gens and done < n:
        try:
            next(gens[0])
            done += 1
        except StopIteration:
            gens.pop(0)


def _exhaust(gens):
    while gens:
        try:
            next(gens[0])
        except StopIteration:
            gens.pop(0)


def build_program():
    nc = bass.Bass("TRN2", target_bir_lowering=False, debug=False)

    def din(name, shape, dt=f32):
        return nc.dram_tensor(name, shape, dt, kind="ExternalInput").ap()

    dc_own_d = din("dc_own", [D, R], bf16)
    dke_d = din("dke", [D, S], bf16)
    enc_d = din("encT", [D, S], bf16)
    mask_d = din("mask128", [128, 128])          # diag causal block [k,q] * -8e9
    padb_d = din("padb", [128, 8])               # -1e9 * padding_mask, chunked
    vm1_d = din("vm1", [128, 8])                 # self V-row mask (drain scale)
    vm1r_d = din("vm1r", [128, 8, 16], bf16)     # ones-row, self
    vm2r_d = din("vm2r", [128, 8, 16], bf16)     # ones-row, cross
    ones_d = din("onesb", [128, 128], bf16)
    sel_d = din("sel128", [128, 1024])           # one-hot head selector
    onesf_d = din("onesf", [128, 128])           # f32 ones
    w_d = {k: din(k, [D, D], bf16)
           for k in ("wq1", "wk1", "wq2", "wk2", "wo1", "wo2")}
    fw1_d = din("fw1", [D, DFF], bf16)
    fw2_d = din("fw2", [DFF, D], bf16)
    bc_d = {k: din(k, [128, 8]) for k in
            ("bq1c", "bk1c", "bq2c", "bk2c", "bo1c", "bo2c", "fb2c",
             "g1c", "b1c", "g2c", "b2c", "g3c", "b3c")}
    fb1c_d = din("fb1c", [128, 32])
    out_d = nc.dram_tensor("outT", [D, R], f32, kind="ExternalOutput").ap()

    with tile.TileContext(nc) as tc:
        with tc.tile_pool(name="persist", bufs=1) as pp, \
             tc.tile_pool(name="consts", bufs=1) as cp:
            arena = pp.tile([128, _TOT], bf16, name="arena")

            def reg_bf(o, n, f):
                return arena[:, o:o + n].rearrange("p (f r) -> p f r", f=f)

            def reg_fr(o, n, f):
                return arena[:, o:o + n].bitcast(f32r).rearrange(
                    "p (f r) -> p f r", f=f)

            dc_own = reg_bf(_A, 4096, 8)             # [128,8,512] bf16
            dke = reg_bf(_B, 8192, 8)                # [128,8,1024] bf16
            kT2 = reg_bf(_B, 8192, 8)
            encv = reg_bf(_C, 8192, 8)
            kT1 = reg_bf(_D, 8192, 8)
            hidden = reg_bf(_C, 16384, 32)           # [128,32,512] bf16
            vaug1 = arena[:, _E:_E + 8320].rearrange(
                "p (f h v) -> p f h v", f=8, h=16)   # [128,8,16,65] bf16
            abufB = reg_bf(_E, 4096, 8)
            vaug2 = arena[:, _F:_F + 8320].rearrange(
                "p (f h v) -> p f h v", f=8, h=16)
            x2 = reg_bf(_F, 4096, 8)
            qbuf1 = reg_bf(_G, 4096, 8)              # [128,8,512] bf16
            qbuf2 = reg_bf(_H, 4096, 8)
            abufA = reg_bf(_I, 4096, 8)
            xat = pp.tile([128, 4096], f32r, name="xat")
            xa = xat.rearrange("p (f r) -> p f r", f=8)

            # ---- constants / small persistents ----
            onesb = cp.tile([128, 128], bf16, name="onesb")
            sel16 = cp.tile([128, 1024], f32r, name="sel16")
            onesbr = cp.tile([128, 128], f32r, name="onesbr")
            mask128 = cp.tile([128, 128], f32, name="mask128")
            padb = cp.tile([128, 8], f32, name="padb")
            vm1 = cp.tile([128, 8], f32, name="vm1")
            bcs = {k: cp.tile([128, 8], f32, name=k) for k in bc_d}
            fb1c = cp.tile([128, 32], f32, name="fb1c")
            den16 = cp.tile([128, 512], f32, name="den16")
            rec16 = cp.tile([128, 512], f32r, name="rec16")

            # startup DMAs: dc_own chunk0 first (warmup+q1), weights stream on
            # sync; dke on scalar; everything else on gpsimd.
            dco_r = dc_own_d.rearrange("(f p) r -> p f r", p=128)
            for kc in range(8):
                nc.sync.dma_start(out=dc_own[:, kc, :], in_=dco_r[:, kc, :])
            dke_r = dke_d.rearrange("(f p) r -> p f r", p=128)
            for kc in range(8):
                nc.scalar.dma_start(out=dke[:, kc, :], in_=dke_r[:, kc, :])
            nc.gpsimd.dma_start(out=bcs["bq1c"], in_=bc_d["bq1c"])
            nc.gpsimd.dma_start(out=bcs["bk1c"], in_=bc_d["bk1c"])
            nc.gpsimd.dma_start(out=onesb, in_=ones_d)
            nc.gpsimd.dma_start(out=sel16, in_=sel_d.bitcast(f32r))
            nc.gpsimd.dma_start(out=onesbr, in_=onesf_d.bitcast(f32r))
            for c in range(4):
                nc.gpsimd.dma_start(out=den16[:, c * 128:(c + 1) * 128],
                                    in_=onesf_d)
            nc.gpsimd.dma_start(out=mask128, in_=mask_d)
            nc.gpsimd.dma_start(out=padb, in_=padb_d)
            nc.gpsimd.dma_start(out=vm1, in_=vm1_d)
            for k in bcs:
                if k not in ("bq1c", "bk1c"):
                    nc.gpsimd.dma_start(out=bcs[k], in_=bc_d[k])
            nc.gpsimd.dma_start(out=fb1c, in_=fb1c_d)
            for rc in range(8):
                nc.gpsimd.dma_start(out=vaug1[:, rc, :, 64:65],
                                    in_=vm1r_d[:, rc, :])
            enc_r = enc_d.rearrange("(f p) r -> p f r", p=128)
            for kc in range(8):
                nc.gpsimd.dma_start(out=encv[:, kc, :], in_=enc_r[:, kc, :])

            ones1 = onesb[0:1, :]      # [1,128] bf16
            onesp = onesb[:, 0:1]      # [128,1] bf16
            ones1r = onesbr[0:1, :]    # [1,128] f32r
            onespr = onesbr[:, 0:1]    # [128,1] f32r

            wall = ctx_wp = tc.tile_pool(name="wall", bufs=12)
            wall = wall.__enter__()

            def lp():
                return nc.allow_low_precision(reason="f32r/bf16 by design")

            # ---------------- generator gemm helpers ----------------
            def gemm_TN(Wd, xt, KCn, MCn, NN, drain, ps, dq):
                """OUT^T[mi, n] = sum_kc W[kc,mi]^T @ xt(kc,n).
                Weight DMAs hoisted to [128, 4*128] chunks shared by 4
                mi-groups; PSUM stays at one bank per (mi, n)."""
                for m4 in range(0, MCn, 4):
                    gs = min(4, MCn - m4)
                    wts = []
                    for kc in range(KCn):
                        wt = wall.tile([128, 512], bf16, name="wt4")
                        dq(out=wt, in_=Wd[kc * 128:(kc + 1) * 128,
                                          m4 * 128:(m4 + gs) * 128])
                        wts.append(wt)
                        if kc % 2 == 1:
                            yield
                    for i in range(gs):
                        pps = [ps.tile([128, 512], f32, name="pp")
                               for _ in range(NN)]
                        for kc in range(KCn):
                            for n in range(NN):
                                nc.tensor.matmul(
                                    pps[n][:],
                                    lhsT=wts[kc][:, i * 128:(i + 1) * 128],
                                    rhs=xt(kc, n),
                                    start=(kc == 0), stop=(kc == KCn - 1))
                            if kc % 2 == 1:
                                yield
                        for n in range(NN):
                            drain(m4 + i, n, pps[n])
                        yield

            def gemm_NT(Wd, xt_sb, KCn, RCn, NFn, drain, ps, dq):
                """OUT[rc] = X @ W; lhsT = xT chunks (stationary), rhs = W
                col tiles, loaded once per (kc, nf) and shared by all rc."""
                for nf in range(NFn):
                    wts = []
                    for kc in range(KCn):
                        wt = wall.tile([128, 512], bf16, name="wtn4")
                        dq(out=wt, in_=Wd[kc * 128:(kc + 1) * 128,
                                          nf * 512:(nf + 1) * 512])
                        wts.append(wt)
                        if kc % 2 == 1:
                            yield
                    for rc in range(RCn):
                        pp = ps.tile([128, 512], f32, name="pp")
                        for kc in range(KCn):
                            nc.tensor.matmul(
                                pp[:],
                                lhsT=xt_sb[:, kc, rc * 128:(rc + 1) * 128],
                                rhs=wts[kc][:],
                                start=(kc == 0), stop=(kc == KCn - 1))
                            if kc % 2 == 1:
                                yield
                        drain(rc, nf, pp)
                        yield

            # ---------------- attention phase ----------------
            def attention(q_sb, kT, va, out_sb, is_self, pulls,
                          ssp, avp, ep, stp):
                kc_order = [0, 1, 2, 3, 7, 6, 5, 4] if is_self else range(8)
                kc_order = list(kc_order)
                for f in range(8):
                    avs = [avp.tile([65, 512], f32, name="av")
                           for _ in range(2)]
                    prev = None
                    for idx, kc in enumerate(kc_order):
                        c0 = (kc - 4) * 128 if (is_self and kc >= 4) else 0
                        sss = [ssp.tile([128, 512], f32, name="ss")
                               for _ in range(2)]
                        for a in range(2):
                            nc.tensor.matmul(
                                sss[a][:, c0:512],
                                lhsT=kT[64 * a:64 * (a + 1), f,
                                        kc * 128:(kc + 1) * 128],
                                rhs=q_sb[64 * a:64 * (a + 1), f, c0:512],
                                start=True, stop=True)
                        if prev is not None:
                            pes, pc0, pkc, pidx = prev
                            for a in range(2):
                                nc.tensor.matmul(
                                    avs[a][:, pc0:512],
                                    lhsT=va[:, pkc, 2 * f + a, :],
                                    rhs=pes[a][:, pc0:512],
                                    start=(pidx == 0), stop=False)
                        if is_self and kc >= 4:
                            for a in range(2):
                                nc.vector.tensor_tensor(
                                    out=sss[a][:, c0:c0 + 128],
                                    in0=sss[a][:, c0:c0 + 128],
                                    in1=mask128[:], op=ALU.add)
                        es = [ep.tile([128, 512], bf16, name="ee")
                              for _ in range(2)]
                        for a in range(2):
                            bias = 0.0 if is_self else padb[:, kc:kc + 1]
                            nc.scalar.activation(es[a][:, c0:512],
                                                 sss[a][:, c0:512],
                                                 AF.Exp, bias=bias, scale=0.125)
                        prev = (es, c0, kc, idx)
                        _pull(pulls, 3)
                    pes, pc0, pkc, pidx = prev
                    for a in range(2):
                        nc.tensor.matmul(avs[a][:, pc0:512],
                                         lhsT=va[:, pkc, 2 * f + a, :],
                                         rhs=pes[a][:, pc0:512],
                                         start=False, stop=True)
                    for a in range(2):
                        h = 2 * f + a
                        stg = stp.tile([1, 512], f32, name="stg")
                        with lp():
                            nc.vector.tensor_scalar_mul(
                                out_sb[64 * a:64 * (a + 1), f, :],
                                avs[a][0:64, :], 1.0)
                            nc.vector.tensor_scalar_mul(
                                stg[:], avs[a][64:65, :], 1.0)
                        nc.gpsimd.dma_start(out=den16[h:h + 1, :], in_=stg[:])
                    _pull(pulls, 2)

            def phase_end(out_sb, bpp):
                with lp():
                    nc.vector.reciprocal(rec16[:], den16[:])
                yield
                for f in range(8):
                    for a in range(2):
                        h = 2 * f + a
                        bpt = bpp.tile([64, 512], f32, name="bp")
                        nc.tensor.matmul(bpt[:],
                                         lhsT=sel16[:, h * 64:(h + 1) * 64],
                                         rhs=rec16[:], start=True, stop=True)
                        with lp():
                            nc.vector.tensor_mul(
                                out_sb[64 * a:64 * (a + 1), f, :],
                                out_sb[64 * a:64 * (a + 1), f, :], bpt[:])
                        yield

            # ---------------- layernorm helpers ----------------
            def ln_stat(x_chunk, xones, pm, pv, first, last, sqp):
                nc.tensor.matmul(pm[:], lhsT=xones, rhs=x_chunk,
                                 start=first, stop=last)
                sq = sqp.tile([128, 512], f32r, name="sq")
                nc.scalar.activation(sq[:], x_chunk, AF.Square)
                nc.tensor.matmul(pv[:], lhsT=onespr, rhs=sq[:],
                                 start=first, stop=last)

            def ln_finalize(x_sb, pm, pv, gC, bC, ltp, bcp):
                """normalize x_sb [128,8,512] in place; yields between steps."""
                m = ltp.tile([1, 512], f32, name="lm")
                sc = ltp.tile([1, 512], f32, name="lsc")
                sc2 = ltp.tile([1, 512], f32, name="lsc2")
                inv = ltp.tile([1, 512], f32r, name="linv")
                minv = ltp.tile([1, 512], f32r, name="lminv")
                nc.vector.tensor_scalar_mul(m[:], pm[:], 1.0 / D)
                nc.vector.tensor_scalar_mul(sc[:], pv[:], 1.0 / D)
                nc.vector.tensor_mul(sc2[:], m[:], m[:])
                nc.vector.tensor_scalar_add(sc2[:], sc2[:], -EPS)
                nc.vector.tensor_tensor(out=sc[:], in0=sc[:], in1=sc2[:],
                                        op=ALU.subtract)
                nc.scalar.activation(sc[:], sc[:], AF.Sqrt)
                with lp():
                    nc.vector.reciprocal(inv[:], sc[:])
                    nc.vector.tensor_mul(minv[:], m[:], inv[:])
                yield
                binv = bcp.tile([128, 512], f32, name="binv")
                bmv = bcp.tile([128, 512], f32, name="bmv")
                nc.tensor.matmul(binv[:], lhsT=ones1r, rhs=inv[:],
                                 start=True, stop=True)
                nc.tensor.matmul(bmv[:], lhsT=ones1r, rhs=minv[:],
                                 start=True, stop=True)
                yield
                for c in range(8):
                    with lp():
                        nc.vector.tensor_tensor(out=x_sb[:, c, :],
                                                in0=x_sb[:, c, :],
                                                in1=binv[:], op=ALU.mult)
                        nc.vector.tensor_tensor(out=x_sb[:, c, :],
                                                in0=x_sb[:, c, :],
                                                in1=bmv[:], op=ALU.subtract)
                    nc.scalar.activation(x_sb[:, c, :], x_sb[:, c, :],
                                         AF.Identity, bias=bC[:, c:c + 1],
                                         scale=gC[:, c:c + 1])
                    yield

            def dummy_fill(ps, n):
                for i in range(n):
                    dt = ps.tile([128, 512], f32, name="pp")
                    nc.tensor.matmul(dt[:], lhsT=onesb[:, 0:128],
                                     rhs=dc_own[:, 0, :], start=True, stop=True)

            # ================= S0: warmup + self projections =================
            with tc.tile_pool(name="warmp", bufs=1, space="PSUM") as wps, \
                 tc.tile_pool(name="ps0", bufs=4, space="PSUM") as ps0:
                wtile = wps.tile([128, 512], f32, name="warm")
                for wi in range(24):
                    nc.tensor.matmul(wtile[:], lhsT=onesb[:, 0:128],
                                     rhs=dc_own[:, 0, :],
                                     start=(wi == 0), stop=(wi == 23))

                def drain_q1(mi, n, pa):
                    nc.scalar.activation(qbuf1[:, mi, :], pa[:], AF.Identity,
                                         bias=bcs["bq1c"][:, mi:mi + 1])
                g = gemm_TN(w_d["wq1"], lambda kc, n: dc_own[:, kc, :],
                            8, 8, 1, drain_q1, ps0, nc.sync.dma_start)
                _exhaust([g])

                def drain_k1(mi, n, pa):
                    nc.scalar.activation(kT1[:, mi, n * 512:(n + 1) * 512],
                                         pa[:], AF.Identity,
                                         bias=bcs["bk1c"][:, mi:mi + 1])
                g = gemm_TN(w_d["wk1"],
                            lambda kc, n: dke[:, kc, n * 512:(n + 1) * 512],
                            8, 8, 2, drain_k1, ps0, nc.sync.dma_start)
                _exhaust([g])

                def drain_v1(rc, nf, pa):
                    dst = vaug1[:, rc, nf * 8:(nf + 1) * 8, 0:64]
                    src = pa[:].rearrange("p (h d) -> p h d", h=8)
                    nc.scalar.activation(dst, src, AF.Copy,
                                         scale=vm1[:, rc:rc + 1])
                g = gemm_NT(w_d["wq1"], dke, 8, 8, 2, drain_v1,
                            ps0, nc.sync.dma_start)
                _exhaust([g])

            # vaug2 ones-rows can land any time before attn2
            for rc in range(8):
                nc.gpsimd.dma_start(out=vaug2[:, rc, :, 64:65],
                                    in_=vm2r_d[:, rc, :])

            # ============ S1 + S2: attention with interleaved gemms ============
            with tc.tile_pool(name="ssp", bufs=2, space="PSUM") as ssp, \
                 tc.tile_pool(name="avp", bufs=2, space="PSUM") as avp, \
                 tc.tile_pool(name="bppA", bufs=2, space="PSUM") as bppA, \
                 tc.tile_pool(name="gemA", bufs=2, space="PSUM") as gemA, \
                 tc.tile_pool(name="ep", bufs=4) as ep, \
                 tc.tile_pool(name="stp", bufs=3) as stp:

                # S1: self attention + cross projections
                def drain_q2(mi, n, pa):
                    nc.scalar.activation(qbuf2[:, mi, :], pa[:], AF.Identity,
                                         bias=bcs["bq2c"][:, mi:mi + 1])

                def drain_k2(mi, n, pa):
                    nc.scalar.activation(kT2[:, mi, n * 512:(n + 1) * 512],
                                         pa[:], AF.Identity,
                                         bias=bcs["bk2c"][:, mi:mi + 1])

                def drain_v2(rc, nf, pa):
                    dst = vaug2[:, rc, nf * 8:(nf + 1) * 8, 0:64]
                    src = pa[:].rearrange("p (h d) -> p h d", h=8)
                    nc.scalar.activation(dst, src, AF.Copy)

                s1_gens = [
                    gemm_TN(w_d["wq2"], lambda kc, n: dc_own[:, kc, :],
                            8, 8, 1, drain_q2, gemA, nc.sync.dma_start),
                    gemm_TN(w_d["wk2"],
                            lambda kc, n: encv[:, kc, n * 512:(n + 1) * 512],
                            8, 8, 2, drain_k2, gemA, nc.sync.dma_start),
                    gemm_NT(w_d["wq2"], encv, 8, 8, 2, drain_v2,
                            gemA, nc.sync.dma_start),
                ]
                attention(qbuf1, kT1, vaug1, abufA, True, s1_gens,
                          ssp, avp, ep, stp)
                _exhaust(s1_gens)

                # S2: cross attention + softmax1 normalize + wo1
                def drain_wo1(mi, n, pa):
                    tw = twp.tile([128, 512], f32, name="tw")
                    nc.scalar.activation(tw[:], pa[:], AF.Identity,
                                         bias=bcs["bo1c"][:, mi:mi + 1])
                    with lp():
                        nc.vector.tensor_add(xa[:, mi, :], tw[:],
                                             dc_own[:, mi, :])

                with tc.tile_pool(name="twp", bufs=2) as twp:
                    s2_gens = [
                        phase_end(abufA, bppA),
                        gemm_TN(w_d["wo1"], lambda kc, n: abufA[:, kc, :],
                                8, 8, 1, drain_wo1, gemA, nc.sync.dma_start),
                    ]
                    attention(qbuf2, kT2, vaug2, abufB, False, s2_gens,
                              ssp, avp, ep, stp)
                    _exhaust(s2_gens)

            # ================= S3a: LN1 || wo2 -> LN2 =================
            with tc.tile_pool(name="bppB", bufs=2, space="PSUM") as bppB, \
                 tc.tile_pool(name="gemB", bufs=2, space="PSUM") as gemB, \
                 tc.tile_pool(name="lnP", bufs=1, space="PSUM") as lnP, \
                 tc.tile_pool(name="bcP", bufs=1, space="PSUM") as bcP, \
                 tc.tile_pool(name="sqp", bufs=2) as sqp, \
                 tc.tile_pool(name="ltp", bufs=1) as ltp, \
                 tc.tile_pool(name="twp2", bufs=2) as twp2:
                # LN1 stats on xa (PE work covering the softmax2 reciprocal)
                pm1 = lnP.tile([1, 512], f32, name="pm")
                pv1 = lnP.tile([1, 512], f32, name="pv")
                for c in range(8):
                    ln_stat(xa[:, c, :], onespr, pm1, pv1, c == 0, c == 7, sqp)
                # softmax2 normalize (recip on DVE; bp matmuls on PE)
                _exhaust([phase_end(abufB, bppB)])
                # LN1 finalize fully (its final ACTs must precede wo2 drains)
                ln1 = ln_finalize(xa, pm1, pv1, bcs["g1c"], bcs["b1c"],
                                  ltp, bcP)
                _exhaust([ln1])
                dummy_fill(gemB, 16)

                # wo2 gemm; drains add x1 residual and feed LN2 stats
                pm2 = lnP.tile([1, 512], f32, name="pm")
                pv2 = lnP.tile([1, 512], f32, name="pv")

                def drain_wo2(mi, n, pa):
                    tw = twp2.tile([128, 512], f32, name="tw2")
                    nc.scalar.activation(tw[:], pa[:], AF.Identity,
                                         bias=bcs["bo2c"][:, mi:mi + 1])
                    with lp():
                        nc.vector.tensor_add(x2[:, mi, :], tw[:], xa[:, mi, :])
                    ln_stat(x2[:, mi, :], onesp, pm2, pv2, mi == 0, mi == 7, sqp)

                _exhaust([gemm_TN(w_d["wo2"], lambda kc, n: abufB[:, kc, :],
                                  8, 8, 1, drain_wo2, gemB,
                                  nc.sync.dma_start)])
                dummy_fill(gemB, 20)
                _exhaust([ln_finalize(x2, pm2, pv2, bcs["g2c"], bcs["b2c"],
                                      ltp, bcP)])

            # ================= S3b: FFN + LN3 + output =================
            with tc.tile_pool(name="gemC", bufs=2, space="PSUM") as gemC, \
                 tc.tile_pool(name="ffnP", bufs=2, space="PSUM") as ffnP, \
                 tc.tile_pool(name="lnP2", bufs=1, space="PSUM") as lnP2, \
                 tc.tile_pool(name="bcP2", bufs=1, space="PSUM") as bcP2, \
                 tc.tile_pool(name="sqp2", bufs=2) as sqp2, \
                 tc.tile_pool(name="ltp2", bufs=1) as ltp2, \
                 tc.tile_pool(name="twp3", bufs=2) as twp3:

                def drain_f1(mi, n, pa):
                    nc.scalar.activation(hidden[:, mi, :], pa[:], AF.Relu,
                                         bias=fb1c[:, mi:mi + 1])
                _exhaust([gemm_TN(fw1_d, lambda kc, n: x2[:, kc, :],
                                  8, 32, 1, drain_f1, gemC,
                                  nc.scalar.dma_start)])

                pm3 = lnP2.tile([1, 512], f32, name="pm3")
                pv3 = lnP2.tile([1, 512], f32, name="pv3")
                # ffn2 in quarters (2 out-chunks each) so PSUM stays at 2 banks
                for qd in range(4):
                    pps = [ffnP.tile([128, 512], f32, name="pf")
                           for _ in range(2)]
                    for kc in range(32):
                        wt = wall.tile([128, 256], bf16, name="wtf")
                        dq = nc.sync.dma_start if kc % 2 == 0 \
                            else nc.gpsimd.dma_start
                        dq(out=wt, in_=fw2_d[kc * 128:(kc + 1) * 128,
                                             qd * 256:(qd + 1) * 256])
                        for i in range(2):
                            nc.tensor.matmul(
                                pps[i][:], lhsT=wt[:, i * 128:(i + 1) * 128],
                                rhs=hidden[:, kc, :],
                                start=(kc == 0), stop=(kc == 31))
                    for i in range(2):
                        mi = qd * 2 + i
                        tw = twp3.tile([128, 512], f32, name="tw3")
                        nc.scalar.activation(tw[:], pps[i][:], AF.Identity,
                                             bias=bcs["fb2c"][:, mi:mi + 1])
                        with lp():
                            nc.vector.tensor_add(xa[:, mi, :], tw[:],
                                                 x2[:, mi, :])
                        ln_stat(xa[:, mi, :], onespr, pm3, pv3, mi == 0, mi == 7, sqp2)

                _exhaust([ln_finalize(xa, pm3, pv3, bcs["g3c"], bcs["b3c"],
                                      ltp2, bcP2)])
                for mi in range(8):
                    nc.sync.dma_start(
                        out=out_d[mi * 128:(mi + 1) * 128, :].bitcast(f32r),
                        in_=xa[:, mi, :])
            ctx_wp.__exit__(None, None, None)

    _split_waits(nc, 1)
    return nc


_PROGRAM = None


def _get_program():
    global _PROGRAM
    if _PROGRAM is None:
        _PROGRAM = build_program()
    return _PROGRAM


def _chunk(a, n):
    return np.ascontiguousarray(np.asarray(a, np.float32).reshape(n, 128).T)


def _shared_inputs(inp):
    """Per-run (not per-core) conversions."""
    sh = {}
    for k in ("wq1", "wk1", "wq2", "wk2", "wo1", "wo2"):
        sh[k] = np.ascontiguousarray(np.asarray(inp[k], np.float32).astype(BF))
    sh["fw1"] = np.ascontiguousarray(np.asarray(inp["ff_w1"], np.float32).astype(BF))
    sh["fw2"] = np.ascontiguousarray(np.asarray(inp["ff_w2"], np.float32).astype(BF))
    wo1 = np.asarray(inp["wo1"], np.float32)
    wo2 = np.asarray(inp["wo2"], np.float32)
    bo1e = np.asarray(inp["bq1"], np.float32) @ wo1 + np.asarray(inp["bo1"], np.float32)
    bo2e = np.asarray(inp["bq2"], np.float32) @ wo2 + np.asarray(inp["bo2"], np.float32)
    la = np.asarray(inp["look_ahead_mask"], np.float32)[0, 0]
    sh["mask128"] = np.ascontiguousarray(la[:128, :128].T) * np.float32(-8e9)
    sh["sel128"] = np.zeros((128, 1024), np.float32)
    for h in range(16):
        sh["sel128"][h, h * 64:(h + 1) * 64] = 1
    sh["onesb"] = np.ones((128, 128), BF)
    sh["onesf"] = np.ones((128, 128), np.float32)
    sh["vm2r"] = np.ones((128, 8, 16), BF)
    sh.update({
        "bq1c": _chunk(inp["bq1"], 8), "bk1c": _chunk(inp["bk1"], 8),
        "bq2c": _chunk(inp["bq2"], 8), "bk2c": _chunk(inp["bk2"], 8),
        "bo1c": _chunk(bo1e, 8), "bo2c": _chunk(bo2e, 8),
        "fb1c": _chunk(inp["ff_b1"], 32), "fb2c": _chunk(inp["ff_b2"], 8),
        "g1c": _chunk(inp["ln1_g"], 8), "b1c": _chunk(inp["ln1_b"], 8),
        "g2c": _chunk(inp["ln2_g"], 8), "b2c": _chunk(inp["ln2_b"], 8),
        "g3c": _chunk(inp["ln3_g"], 8), "b3c": _chunk(inp["ln3_b"], 8),
    })
    return sh


def _core_inputs(inp, sh, c):
    b, j = c // 2, c % 2
    dec = np.asarray(inp["dec_input"], np.float32)[b]        # [S, D]
    enc = np.asarray(inp["enc_output"], np.float32)[b]
    decT = np.ascontiguousarray(dec.T)                       # [D, S]
    own = np.ascontiguousarray(decT[:, j * R:(j + 1) * R])
    if j == 1:
        dkeT = decT                                          # ctx | own
    else:
        dkeT = np.concatenate([decT[:, R:], decT[:, :R]], axis=1)
    padb = (np.asarray(inp["padding_mask"], np.float32)[b, 0, 0]
            * np.float32(-1e9))
    vm = np.ones(S, np.float32)
    if j == 0:
        vm[:R] = 0.0
    vmc = _chunk(vm, 8)
    m = {
        "dc_own": np.ascontiguousarray(own.astype(BF)),
        "dke": np.ascontiguousarray(dkeT.astype(BF)),
        "encT": np.ascontiguousarray(enc.T.astype(BF)),
        "padb": _chunk(padb, 8),
        "vm1": vmc,
        "vm1r": np.ascontiguousarray(
            np.repeat(vmc[:, :, None], 16, axis=2).astype(BF)),
    }
    m.update(sh)
    return m


def kernel(**inputs):
    nc = _get_program()
    sh = _shared_inputs(inputs)
    in_maps = [_core_inputs(inputs, sh, c) for c in range(N_CORES)]
    res = run_bass_kernel_spmd(nc, in_maps, list(range(N_CORES)))
    out = np.empty((B, S, D), np.float32)
    for c in range(N_CORES):
        b, j = c // 2, c % 2
        out[b, j * R:(j + 1) * R, :] = res.results[c]["outT"].T
    return out


if __name__ == "__main__":
    import tempfile
    from concourse.bass_utils import compile_bass_kernel
    nc = build_program()
    with tempfile.TemporaryDirectory() as td:
        compile_bass_kernel(nc, td)
    print("COMPILE OK")
